# revision 29
# baseline (speedup 1.0000x reference)
"""Trainium2 Bass kernel for nn_AttentionProbe_80891414053184.

Math (reference):
    y  = relu(x @ W1.T + b1)            # (B,S,H) -> (B,S,128)
    y2 = relu(y @ W2.T + b2)            # (B,S,128)
    l  = y2 @ Wq.T + pos*pos_w  (+mask) # (B,S,8) logits
    p  = softmax(l, axis=S)
    v  = y2 @ Wv.T + bv
    out[b] = sum_{s,h} p*v + bias       # (B,1)

Strategy: sequence-parallel over 8 cores (512 positions x 4 batches = 2048
tokens per core).  Each core streams its x-shard quantized to fp8-e4m3 on
the host, runs layer 1 as DoubleRow fp8 matmuls (2 contraction rows per PE
cycle -- requires the contraction pair interleaved in the innermost byte
pair of the moving stream), the MLP tail + head projections in f32r, and
emits per-(batch, head) partial softmax sums (Z=sum exp, W=sum exp*v).

Softmax stability without an on-chip max pass: the dominant logit term is
the ALiBi bias pos*pos_w (|pos_w|*4095 can reach ~100), so the host folds
c_h = max(0, pos_w_h*(S-1)) into the additive table; the remaining y2@Wq
part is O(10), safely inside exp's fp32 range.  All cores share the same
offset, so the host merge is a plain sum of Z and W.  bv is folded in on
the host too: sum p*(v+bv) = W/Z + bv.

fp8 numerics: x ~ N(0,1) quantizes to e4m3 directly.  W1 values (~1/64) sit
in e4m3's subnormal range, so the host scales W1 by 64 before quantizing and
folds the 1/64 into W2 (relu is positively homogeneous).  Measured
end-to-end error vs the fp32 reference: ~3.5e-3.

The whole fp8 x-shard (8 MB) fits in SBUF (64 KB/partition), so all x DMAs
are issued up front with no buffer recycling.  Every DMA source is
per-partition contiguous (128 descriptors per transfer), and transfers are
split across the two HWDGE queues (SP + ACT) to halve dispatch
serialization.
"""

import os

import numpy as np

# Problem dims (hardcoded per harness contract).
B, S, H = 4, 4096, 4096
MLP, NH = 128, 8
NCORES = 8
S_SHARD = S // NCORES        # 512 seq positions per core
TOK = B * S_SHARD            # 2048 tokens per core
NT = TOK // 512              # 4 token tiles of 512 (= one batch each)
KCH = H // 128               # 32 contraction chunks of 128
NPAIR = KCH // 2             # 16 DoubleRow pairs
NTAILP = 2                   # pairs delivered per-token-tile at the end
NFULLP = NPAIR - NTAILP      # 14 pairs delivered full-width
GSCHED = [1, 1, 2, 2, 2, 2, 2, 2]   # full-group sizes in pairs (sum=14)
assert sum(GSCHED) == NFULLP
P32 = NT * NH                # 32 packed (tile, head) lanes
# ILV=0: ifmap AP [p, pair, N] (slow-but-safe DoubleRow stream)
# ILV=2: ifmap AP [p, N, pair] with pair step 512 (fast 2-XBUS stream form:
#        checkMatmultPerfMode wants n_elem[2]==2, step%16==0)
ILV = int(os.environ.get("KERNEL_ILV", "2"))
# Second DMA queue: "scalar" (ACT HWDGE), "gpsimd" (SWDGE), "sync" (= all on SP)
QENG = os.environ.get("KERNEL_QENG", "gpsimd")
# SAFE=1: run3-style stats stage (DVE add ca + max pass + separate mul/reduce,
# stats [32,3]) instead of the folded chain -- hang bisect knob.
SAFE = int(os.environ.get("KERNEL_SAFE", "0"))

_cache = {}


def _build_nc():
    import concourse.mybir as mybir
    import concourse.tile as tile
    from concourse import bacc
    from concourse.tile import add_dep_helper

    f32 = mybir.dt.float32
    f32r = mybir.dt.float32r
    fp8 = mybir.dt.float8e4

    nc = bacc.Bacc()
    if ILV == 2:
        # xt9[p, j, t, i, c] = x_shardT[128*(2j+i)+p, 512t+c]: per (pair,
        # tile), element0's 512 cols then element1's -- the matmul rhs AP
        # becomes [p, cols, pair] with pair step 512 (fast 2-XBUS stream).
        xt_d = nc.dram_tensor("xt", [128, NFULLP, NT, 2, 512], fp8,
                              kind="ExternalInput")
    else:
        # xt8[p, j, i, n] = x_shardT[128*(2j+i)+p, n]
        xt_d = nc.dram_tensor("xt", [128, NFULLP, 2, TOK], fp8,
                              kind="ExternalInput")
    # tail pairs, per token tile: xtl[p, t, j, i, c]
    xtl_d = nc.dram_tensor("xtl", [128, NT, NTAILP, 2, 512], fp8,
                           kind="ExternalInput")
    w1_d = nc.dram_tensor("w1s", [128, KCH, MLP], fp8, kind="ExternalInput")
    # cwr: [w2t/64 | wq32 (4 x 32-wide zero-padded blocks) | wv32 | I32] f32r
    CQ = MLP
    CV = MLP + P32 * NT
    CI = MLP + 2 * P32 * NT
    CWRW = CI + P32
    cwr_d = nc.dram_tensor("cwr", [MLP, CWRW], f32r, kind="ExternalInput")
    cb_d = nc.dram_tensor("cb", [MLP, 2], f32, kind="ExternalInput")  # 64b1|b2
    # ca row 8t+h = (batch tile t, head h): pos*pos_w - c_h + mask add
    ca_d = nc.dram_tensor("ca", [P32, 512], f32 if SAFE else f32r,
                          kind="ExternalInput")
    st_d = nc.dram_tensor("stats", [P32, 3 if SAFE else 2], f32,
                          kind="ExternalOutput")

    AF = mybir.ActivationFunctionType
    OP = mybir.AluOpType
    PM = mybir.MatmulPerfMode.DoubleRow

    with tile.TileContext(nc) as tc:
        with (
            tc.tile_pool(name="const", bufs=1) as const,
            tc.tile_pool(name="xp", bufs=len(GSCHED)) as xp,
            tc.tile_pool(name="xlp", bufs=NT) as xlp,
            tc.tile_pool(name="yp", bufs=2) as yp,
            tc.tile_pool(name="y2p", bufs=2) as y2p,
            tc.tile_pool(name="smallp", bufs=1) as smallp,
            tc.tile_pool(name="statsp", bufs=1) as statsp,
            tc.tile_pool(name="ps_y", bufs=4, space="PSUM") as ps_y,
            tc.tile_pool(name="ps_y2", bufs=2, space="PSUM") as ps_y2,
            tc.tile_pool(name="ps_q", bufs=1, space="PSUM") as ps_q,
            tc.tile_pool(name="ps_v", bufs=1, space="PSUM") as ps_v,
        ):
            w1_sb = const.tile([128, KCH, MLP], fp8)
            cwr_sb = const.tile([MLP, CWRW], f32r)
            cb_sb = const.tile([MLP, 2], f32)
            ca_sb = const.tile([P32, 512], f32r)

            # --- DMA issue.  Two HWDGE queues (SP=sync, ACT=scalar), FIFO
            # per queue, balanced ~4.4 MB each.  x group g must land before
            # group g+1 is needed; small first groups shorten the PE ramp.
            x_sb = []
            pbase = []
            p0 = 0
            xshape = ([128, 1, NT, 2, 512] if ILV == 2
                      else [128, 1, 2, TOK])
            for gi, gsz in enumerate(GSCHED):
                shp = list(xshape)
                shp[1] = gsz
                xg = xp.tile(shp, fp8, tag="x", name=f"x{gi}")
                x_sb.append(xg)
                pbase.append(p0)
                p0 += gsz
            xl_sb = [xlp.tile([128, NTAILP, 2, 512], fp8, tag="xl",
                              name=f"xl{t}") for t in range(NT)]

            def xt_slice(p0, p1):
                return (xt_d[:, p0:p1, :, :, :] if ILV == 2
                        else xt_d[:, p0:p1, :, :])

            eng2 = {"scalar": nc.scalar, "gpsimd": nc.gpsimd,
                    "sync": nc.sync}[QENG]
            # sync queue: even x groups, then tail tiles, then stats out
            # 2nd queue: w1 first pair-group, odd x groups, w1 rest, consts
            eng2.dma_start(out=w1_sb[:, 0:4, :], in_=w1_d[:, 0:4, :])
            nc.sync.dma_start(out=x_sb[0][:], in_=xt_slice(0, GSCHED[0]))
            eng2.dma_start(
                out=x_sb[1][:], in_=xt_slice(pbase[1], pbase[1] + GSCHED[1]))
            eng2.dma_start(out=w1_sb[:, 4:KCH, :], in_=w1_d[:, 4:KCH, :])
            eng2.dma_start(out=cwr_sb[:], in_=cwr_d[:])
            eng2.dma_start(out=cb_sb[:], in_=cb_d[:])
            eng2.dma_start(out=ca_sb[:], in_=ca_d[:])
            for gi in range(2, len(GSCHED)):
                eng = nc.sync if gi % 2 == 0 else eng2
                eng.dma_start(
                    out=x_sb[gi][:],
                    in_=xt_slice(pbase[gi], pbase[gi] + GSCHED[gi]))
            for t in range(NT):
                nc.sync.dma_start(out=xl_sb[t][:], in_=xtl_d[:, t, :, :, :])

            stats_sb = statsp.tile([P32, 3 if SAFE else 2], f32)

            # Warmup: PE observes the w1 first-group DMA lane before the real
            # matmuls so steady-state instructions carry at most one new wait.
            warm_ps = ps_y2.tile([128, 512], f32, tag="y2", name="warm_ps")
            warm_pe = nc.tensor.matmul(warm_ps[0:32, 0:64],
                                       w1_sb[:, 0, 0:32], w1_sb[:, 0, 0:64],
                                       start=True, stop=True)

            # Layer 1: yT[t] (128, 512) += (64*W1T)_pair.T @ x_pair, DoubleRow
            # fp8, k-accumulated over the 16 pairs.
            psum_y = [ps_y.tile([128, 512], f32, tag="y", name=f"y_ps{t}")
                      for t in range(NT)]
            def x_rhs(gi, jj, t):
                if ILV == 2:
                    return x_sb[gi][:, jj, t, :, :].rearrange("p i n -> p n i")
                return x_sb[gi][:, jj, :, t * 512:(t + 1) * 512]

            def xl_rhs(t, j):
                if ILV == 2:
                    return xl_sb[t][:, j, :, :].rearrange("p i n -> p n i")
                return xl_sb[t][:, j, :, :]

            for gi, gsz in enumerate(GSCHED):
                for jj in range(gsz):
                    jp = pbase[gi] + jj
                    for t in range(NT):
                        mm = nc.tensor.matmul(
                            psum_y[t][:],
                            w1_sb[:, 2 * jp:2 * jp + 2, :],
                            x_rhs(gi, jj, t),
                            start=(jp == 0), stop=False,
                            perf_mode=PM)
                        if jp == 0 and t == 0:
                            add_dep_helper(mm.ins, warm_pe.ins, sync=False,
                                           reason="warmup before first mm")
            for t in range(NT):
                for j in range(NTAILP):
                    jp = NFULLP + j
                    nc.tensor.matmul(psum_y[t][:],
                                     w1_sb[:, 2 * jp:2 * jp + 2, :],
                                     xl_rhs(t, j),
                                     start=False, stop=(jp == NPAIR - 1),
                                     perf_mode=PM)

            # Tail-lane warmups (before their first real consumers).
            warm_ps2 = ps_y2.tile([128, 512], f32, tag="y2", name="warm_ps2")
            nc.tensor.matmul(warm_ps2[0:NH, 0:NH], cwr_sb[:, 0:NH],
                             cwr_sb[:, 0:NH], start=True, stop=True)
            warm_act = const.tile([MLP, 1], f32)
            nc.scalar.copy(out=warm_act[:], in_=cb_sb[:, 1:2])
            warm_dve = const.tile([MLP, 1], f32)
            nc.vector.tensor_copy(out=warm_dve[:], in_=cb_sb[:, 0:1])

            q32_ps = ps_q.tile([P32, 512], f32, tag="q", name="q32_ps")
            v32_ps = ps_v.tile([P32, 512], f32, tag="v", name="v32_ps")
            if not SAFE:
                # Fold the additive logit table into the q psum via an
                # identity matmul -- runs as soon as ca lands, off the
                # critical tail path.
                nc.tensor.matmul(q32_ps[:], cwr_sb[0:P32, CI:CI + P32],
                                 ca_sb[:], start=True, stop=False)
            for t in range(NT):
                y_sb = yp.tile([128, 512], f32r, tag="ysb", name=f"y_sb{t}")
                # y_sb = relu(psum + 64*b1) = 64*y; the 1/64 is folded into
                # cwr's W2 block.  relu on DVE (add+max) keeps ACT free for
                # relu2/exp.
                nc.vector.tensor_scalar(out=y_sb[:], in0=psum_y[t][:],
                                        scalar1=cb_sb[:, 0:1],
                                        scalar2=0.0, op0=OP.add, op1=OP.max)
                y2_ps = ps_y2.tile([128, 512], f32, tag="y2", name=f"y2_ps{t}")
                nc.tensor.matmul(y2_ps[:], cwr_sb[:, 0:MLP], y_sb[:],
                                 start=True, stop=True)
                y2_sb = y2p.tile([128, 512], f32r, tag="y2sb", name=f"y2_sb{t}")
                nc.scalar.activation(out=y2_sb[:], in_=y2_ps[:], func=AF.Relu,
                                     bias=cb_sb[:, 1:2], scale=1.0)
                # Head projections: the (128, 32) weight block for tile t is
                # zero outside rows 8t..8t+8, so accumulating all 4 tiles into
                # one (32, 512) bank packs q/v as (tile, head) x seq lanes.
                nc.tensor.matmul(q32_ps[:],
                                 cwr_sb[:, CQ + P32 * t:CQ + P32 * (t + 1)],
                                 y2_sb[:], start=SAFE and (t == 0),
                                 stop=(t == NT - 1))
                nc.tensor.matmul(v32_ps[:],
                                 cwr_sb[:, CV + P32 * t:CV + P32 * (t + 1)],
                                 y2_sb[:], start=(t == 0), stop=(t == NT - 1))

            if SAFE:
                AX = mybir.AxisListType
                l_sb = smallp.tile([P32, 512], f32, tag="l", name="l_sb")
                nc.vector.tensor_add(out=l_sb[:], in0=q32_ps[:],
                                     in1=ca_sb[:])
                nc.vector.tensor_reduce(out=stats_sb[:, 0:1], in_=l_sb[:],
                                        axis=AX.X, op=OP.max, negate=True)
                e_sb = smallp.tile([P32, 512], f32, tag="e", name="e_sb")
                nc.scalar.activation(out=e_sb[:], in_=l_sb[:], func=AF.Exp,
                                     bias=stats_sb[:, 0:1], scale=1.0,
                                     accum_out=stats_sb[:, 1:2])
                ev_sb = smallp.tile([P32, 512], f32, tag="ev", name="ev_sb")
                nc.vector.tensor_mul(out=ev_sb[:], in0=e_sb[:], in1=v32_ps[:])
                nc.vector.tensor_reduce(out=stats_sb[:, 2:3], in_=ev_sb[:],
                                        axis=AX.X, op=OP.add)
            else:
                # e = exp(l); stats[:, 0] = Z = sum e (l already offset by
                # -c_h on the host)
                e_sb = smallp.tile([P32, 512], f32, tag="e", name="e_sb")
                nc.scalar.activation(out=e_sb[:], in_=q32_ps[:], func=AF.Exp,
                                     bias=0.0, scale=1.0,
                                     accum_out=stats_sb[:, 0:1])
                # stats[:, 1] = W = sum e*v, fused multiply+reduce on DVE
                ev_sb = smallp.tile([P32, 512], f32, tag="ev", name="ev_sb")
                nc.vector.tensor_tensor_reduce(
                    out=ev_sb[:], in0=e_sb[:], in1=v32_ps[:], scale=1.0,
                    scalar=0.0, op0=OP.mult, op1=OP.add,
                    accum_out=stats_sb[:, 1:2])

            nc.sync.dma_start(out=st_d[:], in_=stats_sb[:])

    nc.finalize()
    return nc


def get_nc():
    if "nc" not in _cache:
        _cache["nc"] = _build_nc()
    return _cache["nc"]


def make_core_inputs(x, mask, W1, b1, W2, b2, Wq, Wv, bv, pos_w, bias):
    """Host-side shard + transpose + fp8 quantize. Returns 8 in_maps."""
    import ml_dtypes
    FP8 = ml_dtypes.float8_e4m3

    # W1 scaled by 64 so its values quantize in e4m3's normal range; the
    # matching 1/64 is folded into W2 below (exact: power of two).
    w1s = np.ascontiguousarray(
        (W1.astype(np.float32) * 64.0).reshape(MLP, KCH, 128)
        .transpose(2, 1, 0)).astype(FP8)

    CQ = MLP
    CV = MLP + P32 * NT
    CI = MLP + 2 * P32 * NT
    cwr = np.zeros((MLP, CI + P32), dtype=np.float32)
    cwr[:, 0:MLP] = W2.T / 64.0
    for t in range(NT):
        cwr[:, CQ + P32 * t + NH * t:CQ + P32 * t + NH * (t + 1)] = Wq.T
        cwr[:, CV + P32 * t + NH * t:CV + P32 * t + NH * (t + 1)] = Wv.T
    cwr[0:P32, CI:CI + P32] = np.eye(P32, dtype=np.float32)
    cb = np.ascontiguousarray(
        np.stack([b1.astype(np.float32) * 64.0,
                  b2.astype(np.float32)], axis=1), dtype=np.float32)

    pos = np.arange(S, dtype=np.float32)
    maskadd = np.where(mask == 0, np.float32(-1e9), np.float32(0.0))  # (B,S)
    # Host-side stability offset: dominant logit term over the FULL sequence.
    c_h = np.maximum(pos_w.astype(np.float32) * (S - 1), 0.0)       # (NH,)

    in_maps = []
    for c in range(NCORES):
        sl = slice(c * S_SHARD, (c + 1) * S_SHARD)
        xT = np.ascontiguousarray(
            x[:, sl, :].transpose(2, 0, 1).reshape(H, TOK)).astype(FP8)
        xr = xT.reshape(NPAIR, 2, 128, TOK)                 # (jp, i, p, n)
        if ILV == 2:
            # [p, jp, t, i, 512]: per (pair, tile) e0's block then e1's
            xt8 = np.ascontiguousarray(
                xr[0:NFULLP].reshape(NFULLP, 2, 128, NT, 512)
                .transpose(2, 0, 3, 1, 4))
        else:
            xt8 = np.ascontiguousarray(xr[0:NFULLP].transpose(2, 0, 1, 3))
        xtl = np.ascontiguousarray(
            xr[NFULLP:].reshape(NTAILP, 2, 128, NT, 512)
            .transpose(2, 3, 0, 1, 4))              # (128, NT, 2, 2, 512)
        ca = np.empty((P32, 512), dtype=np.float32)
        add_ths = (pos_w.astype(np.float32)[None, :, None]
                   * pos[sl][None, None, :]
                   - c_h[None, :, None]
                   + maskadd[:, None, sl])            # (B=NT, NH, 512)
        ca[:, :] = add_ths.reshape(P32, 512)
        in_maps.append({"xt": xt8, "xtl": xtl, "w1s": w1s, "cwr": cwr,
                        "cb": cb, "ca": ca})
    return in_maps


def merge_stats(stats_all, bv, bias):
    """stats_all: (NCORES, 32, 2|3), row 8t+h = (batch t, head h).
    [Z, W] form: all cores share the same per-head logit offset, so the
    merge is a plain sum.  [-m, Z, W] form (SAFE): online-softmax combine.
    bv folds in on the host: sum_s p*(v+bv) = W/Z + bv."""
    ncols = stats_all.shape[-1]
    st = np.asarray(stats_all, dtype=np.float64).reshape(
        NCORES, NT, NH, ncols)
    if ncols == 3:
        m = -st[..., 0]
        M = m.max(axis=0)
        alpha = np.exp(m - M[None])
        Z = (alpha * st[..., 1]).sum(axis=0)
        W = (alpha * st[..., 2]).sum(axis=0)
    else:
        Z = st[..., 0].sum(axis=0)           # (B, NH)
        W = st[..., 1].sum(axis=0)
    out = (W / Z + np.asarray(bv, dtype=np.float64)[None, :]).sum(axis=1)
    return (out[:, None] + np.float64(bias.reshape(1)[0])).astype(np.float32)


def kernel(x, mask, W1, b1, W2, b2, Wq, Wv, bv, pos_w, bias, _trace=False):
    from concourse.bass_utils import run_bass_kernel_spmd

    x = np.asarray(x, dtype=np.float32)
    in_maps = make_core_inputs(x, np.asarray(mask), *(np.asarray(a) for a in
                               (W1, b1, W2, b2, Wq, Wv, bv, pos_w, bias)))
    nc = get_nc()
    res = run_bass_kernel_spmd(nc, in_maps, core_ids=list(range(NCORES)),
                               trace=_trace)
    stats_all = np.stack([r["stats"] for r in res.results])  # (C, 32, 2)
    out = merge_stats(stats_all, np.asarray(bv), np.asarray(bias))
    if _trace:
        kernel.last_result = res
    return out


# revision 33
# speedup vs baseline: 1.2087x; 1.2087x over previous
"""Trainium2 Bass kernel for nn_AttentionProbe_80891414053184.

Math (reference):
    y  = relu(x @ W1.T + b1)            # (B,S,H) -> (B,S,128)
    y2 = relu(y @ W2.T + b2)            # (B,S,128)
    l  = y2 @ Wq.T + pos*pos_w  (+mask) # (B,S,8) logits
    p  = softmax(l, axis=S)
    v  = y2 @ Wv.T + bv
    out[b] = sum_{s,h} p*v + bias       # (B,1)

Strategy: sequence-parallel over 8 cores (512 positions x 4 batches = 2048
tokens per core).  Each core streams its x-shard quantized to fp8-e4m3 on
the host, runs layer 1 as DoubleRow fp8 matmuls (2 contraction rows per PE
cycle -- requires the contraction pair interleaved in the innermost byte
pair of the moving stream), the MLP tail + head projections in f32r, and
emits per-(batch, head) partial softmax sums (Z=sum exp, W=sum exp*v).

Softmax stability without an on-chip max pass: the dominant logit term is
the ALiBi bias pos*pos_w (|pos_w|*4095 can reach ~100), so the host folds
c_h = max(0, pos_w_h*(S-1)) into the additive table; the remaining y2@Wq
part is O(10), safely inside exp's fp32 range.  All cores share the same
offset, so the host merge is a plain sum of Z and W.  bv is folded in on
the host too: sum p*(v+bv) = W/Z + bv.

fp8 numerics: x ~ N(0,1) quantizes to e4m3 directly.  W1 values (~1/64) sit
in e4m3's subnormal range, so the host scales W1 by 64 before quantizing and
folds the 1/64 into W2 (relu is positively homogeneous).  Measured
end-to-end error vs the fp32 reference: ~3.5e-3.

The whole fp8 x-shard (8 MB) fits in SBUF (64 KB/partition), so all x DMAs
are issued up front with no buffer recycling.  Every DMA source is
per-partition contiguous (128 descriptors per transfer), and transfers are
split across the two HWDGE queues (SP + ACT) to halve dispatch
serialization.
"""

import os

import numpy as np

# Problem dims (hardcoded per harness contract).
B, S, H = 4, 4096, 4096
MLP, NH = 128, 8
NCORES = 8
S_SHARD = S // NCORES        # 512 seq positions per core
TOK = B * S_SHARD            # 2048 tokens per core
NT = TOK // 512              # 4 token tiles of 512 (= one batch each)
KCH = H // 128               # 32 contraction chunks of 128
NPAIR = KCH // 2             # 16 DoubleRow pairs
NTAILP = 2                   # pairs delivered per-token-tile at the end
NFULLP = NPAIR - NTAILP      # 14 pairs delivered full-width
GSCHED = [1, 1, 2, 2, 2, 2, 2, 2]   # full-group sizes in pairs (sum=14)
assert sum(GSCHED) == NFULLP
P32 = NT * NH                # 32 packed (tile, head) lanes
# ILV=0: ifmap AP [p, pair, N] -- the BIR-verifier-blessed DoubleRow form
# (second AP dim Num=2, step%16==0).  ILV=2 ([p, N, pair]) is REJECTED by
# the verifier; the kernel is DMA-bound so the stream form doesn't matter.
ILV = int(os.environ.get("KERNEL_ILV", "0"))
# Second DMA queue: "scalar" (ACT HWDGE), "gpsimd" (SWDGE), "sync" (= all on SP)
QENG = os.environ.get("KERNEL_QENG", "gpsimd")
# SAFE=1: run3-style stats stage (DVE add ca + max pass + separate mul/reduce,
# stats [32,3]) instead of the folded chain -- hang bisect knob.
SAFE = int(os.environ.get("KERNEL_SAFE", "0"))
# TTR=1: use the fused tensor_tensor_reduce for W (hang bisect knob).
TTR = int(os.environ.get("KERNEL_TTR", "0"))

_cache = {}


def _build_nc():
    import concourse.mybir as mybir
    import concourse.tile as tile
    from concourse import bacc
    from concourse.tile import add_dep_helper

    f32 = mybir.dt.float32
    f32r = mybir.dt.float32r
    fp8 = mybir.dt.float8e4

    nc = bacc.Bacc()
    if ILV == 2:
        # xt9[p, j, t, i, c] = x_shardT[128*(2j+i)+p, 512t+c]: per (pair,
        # tile), element0's 512 cols then element1's -- the matmul rhs AP
        # becomes [p, cols, pair] with pair step 512 (fast 2-XBUS stream).
        xt_d = nc.dram_tensor("xt", [128, NFULLP, NT, 2, 512], fp8,
                              kind="ExternalInput")
    else:
        # xt8[p, j, i, n] = x_shardT[128*(2j+i)+p, n]
        xt_d = nc.dram_tensor("xt", [128, NFULLP, 2, TOK], fp8,
                              kind="ExternalInput")
    # tail pairs, per token tile: xtl[p, t, j, i, c]
    xtl_d = nc.dram_tensor("xtl", [128, NT, NTAILP, 2, 512], fp8,
                           kind="ExternalInput")
    w1_d = nc.dram_tensor("w1s", [128, KCH, MLP], fp8, kind="ExternalInput")
    # cwr: [w2t/64 | wq32 (4 x 32-wide zero-padded blocks) | wv32 | I32] f32r
    CQ = MLP
    CV = MLP + P32 * NT
    CI = MLP + 2 * P32 * NT
    CWRW = CI + P32
    cwr_d = nc.dram_tensor("cwr", [MLP, CWRW], f32r, kind="ExternalInput")
    cb_d = nc.dram_tensor("cb", [MLP, 2], f32, kind="ExternalInput")  # 64b1|b2
    # ca row 8t+h = (batch tile t, head h): pos*pos_w - c_h + mask add
    ca_d = nc.dram_tensor("ca", [P32, 512], f32 if SAFE else f32r,
                          kind="ExternalInput")
    st_d = nc.dram_tensor("stats", [P32, 3 if SAFE else 2], f32,
                          kind="ExternalOutput")

    AF = mybir.ActivationFunctionType
    OP = mybir.AluOpType
    PM = mybir.MatmulPerfMode.DoubleRow

    with tile.TileContext(nc) as tc:
        with (
            tc.tile_pool(name="const", bufs=1) as const,
            tc.tile_pool(name="xp", bufs=len(GSCHED)) as xp,
            tc.tile_pool(name="xlp", bufs=NT) as xlp,
            tc.tile_pool(name="yp", bufs=2) as yp,
            tc.tile_pool(name="y2p", bufs=2) as y2p,
            tc.tile_pool(name="smallp", bufs=1) as smallp,
            tc.tile_pool(name="statsp", bufs=1) as statsp,
            tc.tile_pool(name="ps_y", bufs=4, space="PSUM") as ps_y,
            tc.tile_pool(name="ps_y2", bufs=2, space="PSUM") as ps_y2,
            tc.tile_pool(name="ps_q", bufs=1, space="PSUM") as ps_q,
            tc.tile_pool(name="ps_v", bufs=1, space="PSUM") as ps_v,
        ):
            w1_sb = const.tile([128, KCH, MLP], fp8)
            cwr_sb = const.tile([MLP, CWRW], f32r)
            cb_sb = const.tile([MLP, 2], f32)
            ca_sb = const.tile([P32, 512], f32r)

            # --- DMA issue.  Two HWDGE queues (SP=sync, ACT=scalar), FIFO
            # per queue, balanced ~4.4 MB each.  x group g must land before
            # group g+1 is needed; small first groups shorten the PE ramp.
            x_sb = []
            pbase = []
            p0 = 0
            xshape = ([128, 1, NT, 2, 512] if ILV == 2
                      else [128, 1, 2, TOK])
            for gi, gsz in enumerate(GSCHED):
                shp = list(xshape)
                shp[1] = gsz
                xg = xp.tile(shp, fp8, tag="x", name=f"x{gi}")
                x_sb.append(xg)
                pbase.append(p0)
                p0 += gsz
            xl_sb = [xlp.tile([128, NTAILP, 2, 512], fp8, tag="xl",
                              name=f"xl{t}") for t in range(NT)]

            def xt_slice(p0, p1):
                return (xt_d[:, p0:p1, :, :, :] if ILV == 2
                        else xt_d[:, p0:p1, :, :])

            eng2 = {"scalar": nc.scalar, "gpsimd": nc.gpsimd,
                    "sync": nc.sync}[QENG]
            # sync queue: ALL x transfers in consumption order (the PE eats
            # groups in k-order; out-of-order arrival stalls it and lets HAM
            # re-throttle).  2nd queue: the small consts, in parallel.
            eng2.dma_start(out=w1_sb[:, 0:4, :], in_=w1_d[:, 0:4, :])
            for gi in range(len(GSCHED)):
                nc.sync.dma_start(
                    out=x_sb[gi][:],
                    in_=xt_slice(pbase[gi], pbase[gi] + GSCHED[gi]))
                if gi == 0:
                    eng2.dma_start(out=w1_sb[:, 4:KCH, :],
                                   in_=w1_d[:, 4:KCH, :])
                    eng2.dma_start(out=cwr_sb[:], in_=cwr_d[:])
                    eng2.dma_start(out=cb_sb[:], in_=cb_d[:])
                    eng2.dma_start(out=ca_sb[:], in_=ca_d[:])
            for t in range(NT):
                nc.sync.dma_start(out=xl_sb[t][:], in_=xtl_d[:, t, :, :, :])

            stats_sb = statsp.tile([P32, 3 if SAFE else 2], f32)

            # Warmup: PE observes the w1 first-group DMA lane before the real
            # matmuls so steady-state instructions carry at most one new wait.
            warm_ps = ps_y2.tile([128, 512], f32, tag="y2", name="warm_ps")
            warm_pe = nc.tensor.matmul(warm_ps[0:32, 0:64],
                                       w1_sb[:, 0, 0:32], w1_sb[:, 0, 0:64],
                                       start=True, stop=True)

            # Layer 1: yT[t] (128, 512) += (64*W1T)_pair.T @ x_pair, DoubleRow
            # fp8, k-accumulated over the 16 pairs.
            psum_y = [ps_y.tile([128, 512], f32, tag="y", name=f"y_ps{t}")
                      for t in range(NT)]
            def x_rhs(gi, jj, t):
                if ILV == 2:
                    return x_sb[gi][:, jj, t, :, :].rearrange("p i n -> p n i")
                return x_sb[gi][:, jj, :, t * 512:(t + 1) * 512]

            def xl_rhs(t, j):
                if ILV == 2:
                    return xl_sb[t][:, j, :, :].rearrange("p i n -> p n i")
                return xl_sb[t][:, j, :, :]

            for gi, gsz in enumerate(GSCHED):
                for jj in range(gsz):
                    jp = pbase[gi] + jj
                    for t in range(NT):
                        mm = nc.tensor.matmul(
                            psum_y[t][:],
                            w1_sb[:, 2 * jp:2 * jp + 2, :],
                            x_rhs(gi, jj, t),
                            start=(jp == 0), stop=False,
                            perf_mode=PM)
                        if jp == 0 and t == 0:
                            add_dep_helper(mm.ins, warm_pe.ins, sync=False,
                                           reason="warmup before first mm")
            for t in range(NT):
                for j in range(NTAILP):
                    jp = NFULLP + j
                    nc.tensor.matmul(psum_y[t][:],
                                     w1_sb[:, 2 * jp:2 * jp + 2, :],
                                     xl_rhs(t, j),
                                     start=False, stop=(jp == NPAIR - 1),
                                     perf_mode=PM)

            # Tail-lane warmups (before their first real consumers).
            warm_ps2 = ps_y2.tile([128, 512], f32, tag="y2", name="warm_ps2")
            nc.tensor.matmul(warm_ps2[0:NH, 0:NH], cwr_sb[:, 0:NH],
                             cwr_sb[:, 0:NH], start=True, stop=True)
            warm_act = const.tile([MLP, 1], f32)
            nc.scalar.copy(out=warm_act[:], in_=cb_sb[:, 1:2])
            warm_dve = const.tile([MLP, 1], f32)
            nc.vector.tensor_copy(out=warm_dve[:], in_=cb_sb[:, 0:1])

            q32_ps = ps_q.tile([P32, 512], f32, tag="q", name="q32_ps")
            v32_ps = ps_v.tile([P32, 512], f32, tag="v", name="v32_ps")
            if not SAFE:
                # Fold the additive logit table into the q psum via an
                # identity matmul -- runs as soon as ca lands, off the
                # critical tail path.
                nc.tensor.matmul(q32_ps[:], cwr_sb[0:P32, CI:CI + P32],
                                 ca_sb[:], start=True, stop=False)
            for t in range(NT):
                y_sb = yp.tile([128, 512], f32r, tag="ysb", name=f"y_sb{t}")
                # y_sb = relu(psum + 64*b1) = 64*y; the 1/64 is folded into
                # cwr's W2 block.  relu on DVE (add+max) keeps ACT free for
                # relu2/exp.
                nc.vector.tensor_scalar(out=y_sb[:], in0=psum_y[t][:],
                                        scalar1=cb_sb[:, 0:1],
                                        scalar2=0.0, op0=OP.add, op1=OP.max)
                y2_ps = ps_y2.tile([128, 512], f32, tag="y2", name=f"y2_ps{t}")
                nc.tensor.matmul(y2_ps[:], cwr_sb[:, 0:MLP], y_sb[:],
                                 start=True, stop=True)
                y2_sb = y2p.tile([128, 512], f32r, tag="y2sb", name=f"y2_sb{t}")
                nc.scalar.activation(out=y2_sb[:], in_=y2_ps[:], func=AF.Relu,
                                     bias=cb_sb[:, 1:2], scale=1.0)
                # Head projections: the (128, 32) weight block for tile t is
                # zero outside rows 8t..8t+8, so accumulating all 4 tiles into
                # one (32, 512) bank packs q/v as (tile, head) x seq lanes.
                nc.tensor.matmul(q32_ps[:],
                                 cwr_sb[:, CQ + P32 * t:CQ + P32 * (t + 1)],
                                 y2_sb[:], start=bool(SAFE) and t == 0,
                                 stop=(t == NT - 1))
                nc.tensor.matmul(v32_ps[:],
                                 cwr_sb[:, CV + P32 * t:CV + P32 * (t + 1)],
                                 y2_sb[:], start=(t == 0), stop=(t == NT - 1))

            if SAFE:
                AX = mybir.AxisListType
                l_sb = smallp.tile([P32, 512], f32, tag="l", name="l_sb")
                nc.vector.tensor_add(out=l_sb[:], in0=q32_ps[:],
                                     in1=ca_sb[:])
                nc.vector.tensor_reduce(out=stats_sb[:, 0:1], in_=l_sb[:],
                                        axis=AX.X, op=OP.max, negate=True)
                e_sb = smallp.tile([P32, 512], f32, tag="e", name="e_sb")
                nc.scalar.activation(out=e_sb[:], in_=l_sb[:], func=AF.Exp,
                                     bias=stats_sb[:, 0:1], scale=1.0,
                                     accum_out=stats_sb[:, 1:2])
                ev_sb = smallp.tile([P32, 512], f32, tag="ev", name="ev_sb")
                nc.vector.tensor_mul(out=ev_sb[:], in0=e_sb[:], in1=v32_ps[:])
                nc.vector.tensor_reduce(out=stats_sb[:, 2:3], in_=ev_sb[:],
                                        axis=AX.X, op=OP.add)
            else:
                # e = exp(l); stats[:, 0] = Z = sum e (l already offset by
                # -c_h on the host)
                e_sb = smallp.tile([P32, 512], f32, tag="e", name="e_sb")
                nc.scalar.activation(out=e_sb[:], in_=q32_ps[:], func=AF.Exp,
                                     bias=0.0, scale=1.0,
                                     accum_out=stats_sb[:, 0:1])
                # stats[:, 1] = W = sum e*v
                ev_sb = smallp.tile([P32, 512], f32, tag="ev", name="ev_sb")
                if TTR:
                    nc.vector.tensor_tensor_reduce(
                        out=ev_sb[:], in0=e_sb[:], in1=v32_ps[:], scale=1.0,
                        scalar=0.0, op0=OP.mult, op1=OP.add,
                        accum_out=stats_sb[:, 1:2])
                else:
                    AX = mybir.AxisListType
                    nc.vector.tensor_mul(out=ev_sb[:], in0=e_sb[:],
                                         in1=v32_ps[:])
                    nc.vector.tensor_reduce(out=stats_sb[:, 1:2],
                                            in_=ev_sb[:], axis=AX.X,
                                            op=OP.add)

            nc.sync.dma_start(out=st_d[:], in_=stats_sb[:])

    nc.finalize()
    return nc


def get_nc():
    if "nc" not in _cache:
        _cache["nc"] = _build_nc()
    return _cache["nc"]


def make_core_inputs(x, mask, W1, b1, W2, b2, Wq, Wv, bv, pos_w, bias):
    """Host-side shard + transpose + fp8 quantize. Returns 8 in_maps."""
    import ml_dtypes
    FP8 = ml_dtypes.float8_e4m3

    # W1 scaled by 64 so its values quantize in e4m3's normal range; the
    # matching 1/64 is folded into W2 below (exact: power of two).
    w1s = np.ascontiguousarray(
        (W1.astype(np.float32) * 64.0).reshape(MLP, KCH, 128)
        .transpose(2, 1, 0)).astype(FP8)

    CQ = MLP
    CV = MLP + P32 * NT
    CI = MLP + 2 * P32 * NT
    cwr = np.zeros((MLP, CI + P32), dtype=np.float32)
    cwr[:, 0:MLP] = W2.T / 64.0
    for t in range(NT):
        cwr[:, CQ + P32 * t + NH * t:CQ + P32 * t + NH * (t + 1)] = Wq.T
        cwr[:, CV + P32 * t + NH * t:CV + P32 * t + NH * (t + 1)] = Wv.T
    cwr[0:P32, CI:CI + P32] = np.eye(P32, dtype=np.float32)
    cb = np.ascontiguousarray(
        np.stack([b1.astype(np.float32) * 64.0,
                  b2.astype(np.float32)], axis=1), dtype=np.float32)

    pos = np.arange(S, dtype=np.float32)
    maskadd = np.where(mask == 0, np.float32(-1e9), np.float32(0.0))  # (B,S)
    # Host-side stability offset: dominant logit term over the FULL sequence.
    c_h = np.maximum(pos_w.astype(np.float32) * (S - 1), 0.0)       # (NH,)

    in_maps = []
    for c in range(NCORES):
        sl = slice(c * S_SHARD, (c + 1) * S_SHARD)
        xT = np.ascontiguousarray(
            x[:, sl, :].transpose(2, 0, 1).reshape(H, TOK)).astype(FP8)
        xr = xT.reshape(NPAIR, 2, 128, TOK)                 # (jp, i, p, n)
        if ILV == 2:
            # [p, jp, t, i, 512]: per (pair, tile) e0's block then e1's
            xt8 = np.ascontiguousarray(
                xr[0:NFULLP].reshape(NFULLP, 2, 128, NT, 512)
                .transpose(2, 0, 3, 1, 4))
        else:
            xt8 = np.ascontiguousarray(xr[0:NFULLP].transpose(2, 0, 1, 3))
        xtl = np.ascontiguousarray(
            xr[NFULLP:].reshape(NTAILP, 2, 128, NT, 512)
            .transpose(2, 3, 0, 1, 4))              # (128, NT, 2, 2, 512)
        ca = np.empty((P32, 512), dtype=np.float32)
        add_ths = (pos_w.astype(np.float32)[None, :, None]
                   * pos[sl][None, None, :]
                   - c_h[None, :, None]
                   + maskadd[:, None, sl])            # (B=NT, NH, 512)
        ca[:, :] = add_ths.reshape(P32, 512)
        in_maps.append({"xt": xt8, "xtl": xtl, "w1s": w1s, "cwr": cwr,
                        "cb": cb, "ca": ca})
    return in_maps


def merge_stats(stats_all, bv, bias):
    """stats_all: (NCORES, 32, 2|3), row 8t+h = (batch t, head h).
    [Z, W] form: all cores share the same per-head logit offset, so the
    merge is a plain sum.  [-m, Z, W] form (SAFE): online-softmax combine.
    bv folds in on the host: sum_s p*(v+bv) = W/Z + bv."""
    ncols = stats_all.shape[-1]
    st = np.asarray(stats_all, dtype=np.float64).reshape(
        NCORES, NT, NH, ncols)
    if ncols == 3:
        m = -st[..., 0]
        M = m.max(axis=0)
        alpha = np.exp(m - M[None])
        Z = (alpha * st[..., 1]).sum(axis=0)
        W = (alpha * st[..., 2]).sum(axis=0)
    else:
        Z = st[..., 0].sum(axis=0)           # (B, NH)
        W = st[..., 1].sum(axis=0)
    out = (W / Z + np.asarray(bv, dtype=np.float64)[None, :]).sum(axis=1)
    return (out[:, None] + np.float64(bias.reshape(1)[0])).astype(np.float32)


def kernel(x, mask, W1, b1, W2, b2, Wq, Wv, bv, pos_w, bias, _trace=False):
    from concourse.bass_utils import run_bass_kernel_spmd

    x = np.asarray(x, dtype=np.float32)
    in_maps = make_core_inputs(x, np.asarray(mask), *(np.asarray(a) for a in
                               (W1, b1, W2, b2, Wq, Wv, bv, pos_w, bias)))
    nc = get_nc()
    res = run_bass_kernel_spmd(nc, in_maps, core_ids=list(range(NCORES)),
                               trace=_trace)
    stats_all = np.stack([r["stats"] for r in res.results])  # (C, 32, 2)
    out = merge_stats(stats_all, np.asarray(bv), np.asarray(bias))
    if _trace:
        kernel.last_result = res
    return out
